# revision 6
# baseline (speedup 1.0000x reference)
"""GRU GenRNN Trainium2 kernel v2 (nn_C_eAR_GenRNN).

Data-parallel over batch (B=32 -> 4 seq/core). Per core:
  Phase A: gates_x = [onehot(idx) | cnd] @ [G_tab | WcT] (PE GEMM bf16).
           rz columns scaled by S_TOT (absorbed into recurrence psum);
           n columns UNSCALED (consumed on DVE pre-tanh).
  Phase R: sequential GRU scan. W_hh@h in fp8e4m3 DoubleRow (weights
           moving, hT stationary). Post-matmul chain in fp16; hidden
           state kept gate-major [128, chunk, batch] so the h-update
           runs on all 128 lanes; small z/n tensors transposed instead
           of h. Dummy PE matmuls keep the tensor engine p-state warm.
  Phase C: hidden = relu(o_rnn @ fc1.T + b); out = hidden @ fc2.T + b.
           Reads h pre-transposed from DRAM (no transposes).
"""
import numpy as np
import ml_dtypes

import concourse.bass as bass
import concourse.bacc as bacc
import concourse.tile as tile
from concourse import mybir
from concourse.bass_utils import run_bass_kernel_spmd

BF16 = ml_dtypes.bfloat16
NP8 = ml_dtypes.float8_e4m3
NP16 = np.float16
F32 = mybir.dt.float32
F16 = mybir.dt.float16
BF = mybir.dt.bfloat16
FP8 = mybir.dt.float8e4

B, T_FULL, H, EMB, C2, O, FCD = 32, 4096, 896, 256, 512, 512, 896
NCORES, BL = 8, 4
THREEH = 3 * H
NCH = H // 128   # 7 chunks of 128 hidden units
NPAIR = 4        # fp8 DoubleRow pairs: H padded 896 -> 1024 = 4 x 256
RZW = 2 * H      # 1792 (r|z block)
GPAD = 256       # psum pad so the n block starts bank-aligned at 2048
PW = THREEH + GPAD  # 2944: psum/whhp column layout [rz | pad | n]

S_W = 256.0      # W_hh scale into fp8e4m3 normal range
S_H = 64.0       # h scale into fp8e4m3 normal range
S_TOT = S_W * S_H
INV_S = 1.0 / S_TOT

Sig = mybir.ActivationFunctionType.Sigmoid
Tanh = mybir.ActivationFunctionType.Tanh
Relu = mybir.ActivationFunctionType.Relu
Ident = mybir.ActivationFunctionType.Identity
DR = mybir.MatmulPerfMode.DoubleRow


def _mm_windows(lo, hi):
    """Bank-aligned (<=512, non-straddling) windows covering [lo, hi)."""
    out = []
    pos = lo
    while pos < hi:
        end = min(hi, (pos // 512 + 1) * 512)
        out.append((pos, end - pos))
        pos = end
    return out


W_RZ = _mm_windows(0, RZW)          # (0,512)(512,512)(1024,512)(1536,256)
W_N = _mm_windows(RZW + GPAD, PW)   # (2048,512)(2560,384)
W_ALL = W_RZ + W_N


def build_nc(T=T_FULL, unroll=8, dumw=0):
    RT = T * BL
    nc = bacc.Bacc("TRN2", target_bir_lowering=False, debug=False,
                   num_devices=NCORES)
    ein = lambda n, s, d: nc.dram_tensor(n, s, d, kind="ExternalInput").ap()
    idxb_d = ein("idxb", [128, RT], F32)
    cndr_d = ein("cndr", [RT, C2], F32)
    wstack_d = ein("wstack", [1024, THREEH], BF)
    whhp_d = ein("whhp", [NPAIR * 128, 2 * PW], FP8)
    fc1wt_d = ein("fc1wt", [H, FCD], F16)
    fc2wt_d = ein("fc2wt", [FCD, O], F16)
    fc1bt_d = ein("fc1bt", [128, FCD // 128], F32)
    fc2bt_d = ein("fc2bt", [128, O // 128], F32)
    eye4h_d = ein("eye4h", [BL, BL], F16)
    eye416_d = ein("eye416", [BL, 16], BF)
    eye128f_d = ein("eye128f", [128, 128], F32)
    gx_d = nc.dram_tensor("gx_i", [RT, THREEH], BF).ap()
    BLK = BL * unroll
    hT_d = nc.dram_tensor("hT_i", [128, NCH, RT], F16).ap()
    out_d = nc.dram_tensor("outp", [RT, O], F32, kind="ExternalOutput").ap()

    with tile.TileContext(nc) as tc:
        # ---------------- Phase A: gates_x GEMM ----------------
        with (
            tc.tile_pool(name="wA", bufs=1) as wA,
            tc.tile_pool(name="pa", bufs=3) as pa,
            tc.tile_pool(name="cstA", bufs=1) as cstA,
            tc.tile_pool(name="psA", bufs=6, space="PSUM") as psA,
            tc.tile_pool(name="psTA", bufs=2, space="PSUM") as psTA,
        ):
            eye128f = cstA.tile([128, 128], F32)
            nc.sync.dma_start(eye128f[:], eye128f_d[:])
            wstack_sb = []
            for k in range(8):
                wt = wA.tile([128, THREEH], BF, tag=f"wst{k}", name=f"wst{k}")
                nc.sync.dma_start(wt[:], wstack_d[k * 128:(k + 1) * 128, :])
                wstack_sb.append(wt)
            it32 = cstA.tile([128, 1], mybir.dt.int32)
            nc.gpsimd.iota(it32[:], pattern=[[1, 1]], base=0, channel_multiplier=1)
            ocs = []
            for k in range(4):
                oc = cstA.tile([128, 1], F32, tag=f"oc{k}", name=f"oc{k}")
                nc.vector.tensor_scalar_add(oc[:], it32[:], float(128 * k))
                ocs.append(oc)

            for rt in range(RT // 128):
                idxt = pa.tile([128, 128], F32)
                nc.sync.dma_start(idxt[:], idxb_d[:, rt * 128:(rt + 1) * 128])
                xT = []
                for k in range(4):
                    oh = pa.tile([128, 128], BF, tag=f"oh{k}", name=f"oh{k}")
                    nc.vector.tensor_scalar(oh[:], idxt[:], ocs[k][:], None,
                                            op0=mybir.AluOpType.is_equal)
                    xT.append(oh)
                for k in range(4):
                    ct = pa.tile([128, 128], F32, tag=f"ct{k}", name=f"ct{k}")
                    nc.sync.dma_start(
                        ct[:], cndr_d[rt * 128:(rt + 1) * 128,
                                      k * 128:(k + 1) * 128])
                    pst = psTA.tile([128, 128], F32)
                    nc.tensor.transpose(pst[:], ct[:], eye128f[:])
                    cb = pa.tile([128, 128], BF, tag=f"cb{k}", name=f"cb{k}")
                    nc.vector.tensor_copy(cb[:], pst[:])
                    xT.append(cb)
                gxall = pa.tile([128, THREEH], BF, tag="gxall", name="gxall")
                for (w0, wl) in _mm_windows(0, THREEH):
                    pg = psA.tile([128, 512], F32, tag="pgA", name="pgA")
                    for k in range(8):
                        nc.tensor.matmul(pg[:, :wl], xT[k][:],
                                         wstack_sb[k][:, w0:w0 + wl],
                                         start=(k == 0), stop=(k == 7))
                    nc.vector.tensor_copy(gxall[:, w0:w0 + wl], pg[:, :wl])
                nc.sync.dma_start(gx_d[rt * 128:(rt + 1) * 128, :], gxall[:])

        # ---------------- Phase R: GRU recurrence ----------------
        with (
            tc.tile_pool(name="wR", bufs=1) as wR,
            tc.tile_pool(name="stR", bufs=1) as stR,
            tc.tile_pool(name="pr", bufs=3) as pr,
            tc.tile_pool(name="prg", bufs=4) as prg,
            tc.tile_pool(name="prr", bufs=2) as prr,
            tc.tile_pool(name="psR", bufs=1, space="PSUM") as psR,
            tc.tile_pool(name="psT", bufs=1, space="PSUM") as psT,
            tc.tile_pool(name="psD", bufs=1, space="PSUM") as psD,
        ):
            whhp_sb = []
            for p in range(NPAIR):
                wt = wR.tile([128, 2, PW], FP8, tag=f"whp{p}", name=f"whp{p}")
                nc.sync.dma_start(wt[:], whhp_d[p * 128:(p + 1) * 128, :])
                whhp_sb.append(wt)
            eye4h = wR.tile([BL, BL], F16, tag="eye4h")
            nc.sync.dma_start(eye4h[:], eye4h_d[:])
            eye416 = wR.tile([BL, 16], BF, tag="eye416")
            nc.sync.dma_start(eye416[:], eye416_d[:])
            dum8 = wR.tile([128, 2, 16], FP8, tag="dum8")
            nc.vector.memset(dum8[:], 0.0)
            # gate-major h state: hcarry [128, 8, 4] fp16 (chunk 7 = pad, 0)
            hcarry = stR.tile([128, NPAIR * 2, BL], F16, tag="hcarry")
            nc.vector.memset(hcarry[:], 0.0)
            # fp8 stationary for the DR matmul: [128, pair, sub, 16]
            hT8 = stR.tile([128, NPAIR, 2, 16], FP8, tag="hT8")
            nc.vector.memset(hT8[:], 0.0)
            # z / pre-tanh n transposed into psum: [128, {z,nn}, chunk(8), b]
            # (fp16 psum memset is illegal; zero pad chunk 7 via transposes)
            ptr = psT.tile([128, 2, NPAIR * 2, BL], F16, tag="ptr")
            zpad = stR.tile([BL, 128], F16, tag="zpad")
            nc.vector.memset(zpad[:], 0.0)
            for zn in range(2):
                nc.tensor.transpose(ptr[:, zn, NCH, :], zpad[:], eye4h[:])

            def step(row0, ring, u, prev):
                gxb = prg.tile([BL, THREEH], BF, tag="gxb", name="gxb")
                nc.sync.dma_start(gxb[:], gx_d[row0, :])
                pg = psR.tile([16, RZW + GPAD], F32, tag="pg", name="pg")
                pgn = psR.tile([16, PW - RZW - GPAD], F32, tag="pgn",
                               name="pgn")
                # dummy matmuls: independent PE work to keep p-state warm
                for d in range(dumw):
                    pd = psD.tile([16, 512], F32, tag="pd", name="pd")
                    nc.tensor.matmul(pd[:], dum8[:],
                                     whhp_sb[d % NPAIR][:, :, 0:512],
                                     start=True, stop=True, perf_mode=DR)
                # gx(rz) into psum first (independent of hT8)
                for (w0, wl) in W_RZ:
                    nc.tensor.matmul(pg[:, w0:w0 + wl], eye416[:],
                                     gxb[:, w0:w0 + wl],
                                     start=True, stop=False)
                # DR passes: weights moving, hT8 stationary
                for p in range(NPAIR):
                    for (w0, wl) in W_RZ:
                        nc.tensor.matmul(
                            pg[:, w0:w0 + wl], hT8[:, p, :, :],
                            whhp_sb[p][:, :, w0:w0 + wl],
                            start=False,
                            stop=(p == NPAIR - 1), perf_mode=DR)
                    for (w0, wl) in W_N:
                        nc.tensor.matmul(
                            pgn[:, w0 - RZW - GPAD:w0 - RZW - GPAD + wl],
                            hT8[:, p, :, :],
                            whhp_sb[p][:, :, w0:w0 + wl],
                            start=(p == 0),
                            stop=(p == NPAIR - 1), perf_mode=DR)
                # sigmoid(rz) -> fp16 batch-major
                rz16 = pr.tile([BL, RZW], F16, tag="rz16", name="rz16")
                nc.scalar.activation(rz16[:], pg[0:BL, 0:RZW], Sig,
                                     scale=INV_S)
                # raw hn -> fp16 (parallel on DVE)
                hn16 = pr.tile([BL, H], F16, tag="hn16", name="hn16")
                nc.vector.tensor_scalar_mul(hn16[:], pgn[0:BL, :], INV_S)
                rnt = pr.tile([BL, H], F16, tag="rnt", name="rnt")
                nc.vector.tensor_mul(rnt[:], rz16[:, 0:H], hn16[:])
                nnt = pr.tile([BL, H], F16, tag="nnt", name="nnt")
                nc.vector.tensor_add(nnt[:], rnt[:], gxb[:, RZW:THREEH])
                # transpose z and pre-tanh n into ptr (batch -> gate major)
                for c in range(NCH):
                    nc.tensor.transpose(ptr[:, 0, c, :],
                                        rz16[:, H + c * 128:H + (c + 1) * 128],
                                        eye4h[:])
                for c in range(NCH):
                    nc.tensor.transpose(ptr[:, 1, c, :],
                                        nnt[:, c * 128:(c + 1) * 128],
                                        eye4h[:])
                # gate-major tanh on all 128 lanes
                nbT = pr.tile([128, NPAIR * 2, BL], F16, tag="nbT", name="nbT")
                nc.scalar.activation(nbT[:], ptr[:, 1, :, :], Tanh)
                # gate-major h update: h' = n + z*(h - n)
                dhn = pr.tile([128, NPAIR * 2, BL], F16, tag="dhn", name="dhn")
                nc.vector.tensor_sub(dhn[:], prev[:], nbT[:])
                zd = pr.tile([128, NPAIR * 2, BL], F16, tag="zd", name="zd")
                nc.vector.tensor_mul(zd[:], ptr[:, 0, :, :], dhn[:])
                cur = ring[:, :, u, :]
                nc.vector.tensor_add(cur, nbT[:], zd[:])
                # fp8 stationary for next step
                nc.vector.tensor_scalar_mul(
                    hT8[:, :, :, 0:BL],
                    cur.rearrange("p (a b) c -> p a b c", b=2), S_H)
                return cur

            with tc.For_i(0, RT, BLK) as ivr:
                ring = prr.tile([128, NPAIR * 2, unroll, BL], F16,
                                tag="ring", name="ring")
                prev = hcarry
                for u in range(unroll):
                    prev = step(bass.ds(ivr + BL * u, BL), ring, u, prev)
                nc.vector.tensor_copy(hcarry[:], prev)
                # one h-block store (64B/partition contiguous runs per chunk)
                nc.sync.dma_start(
                    hT_d[:, :, bass.ds(ivr, BLK)],
                    ring[:, 0:NCH, :, :].rearrange("p c u b -> p c (u b)"))

        # ---------------- Phase C: FC layers ----------------
        with (
            tc.tile_pool(name="wC", bufs=1) as wC,
            tc.tile_pool(name="pcp", bufs=2) as pcp,
            tc.tile_pool(name="psC1", bufs=2, space="PSUM") as psC1,
            tc.tile_pool(name="psC2", bufs=2, space="PSUM") as psC2,
            tc.tile_pool(name="psTC", bufs=2, space="PSUM") as psTC,
        ):
            eye128fc = wC.tile([128, 128], F32, tag="eye128fc")
            nc.sync.dma_start(eye128fc[:], eye128f_d[:])
            fc1w_sb, fc2w_sb = [], []
            for k in range(NCH):
                wt = wC.tile([128, FCD], F16, tag=f"fc1w{k}", name=f"fc1w{k}")
                nc.sync.dma_start(wt[:], fc1wt_d[k * 128:(k + 1) * 128, :])
                fc1w_sb.append(wt)
                wt2 = wC.tile([128, O], F16, tag=f"fc2w{k}", name=f"fc2w{k}")
                nc.sync.dma_start(wt2[:], fc2wt_d[k * 128:(k + 1) * 128, :])
                fc2w_sb.append(wt2)
            fc1b_sb = wC.tile([128, FCD // 128], F32, tag="fc1b")
            nc.sync.dma_start(fc1b_sb[:], fc1bt_d[:])
            fc2b_sb = wC.tile([128, O // 128], F32, tag="fc2b")
            nc.sync.dma_start(fc2b_sb[:], fc2bt_d[:])

            n_rc = RT // 512
            for rc in range(n_rc):
                r0 = rc * 512
                oT = []
                for fi in range(NCH):
                    ot = pcp.tile([128, 512], F16, tag=f"oT{fi}", name=f"oT{fi}")
                    nc.sync.dma_start(ot[:], hT_d[:, fi, r0:r0 + 512])
                    oT.append(ot)
                hid = []
                for mi in range(NCH):
                    h1 = psC1.tile([128, 512], F32, tag="h1", name="h1")
                    for ki in range(NCH):
                        nc.tensor.matmul(
                            h1[:], fc1w_sb[ki][:, mi * 128:(mi + 1) * 128],
                            oT[ki][:], start=(ki == 0), stop=(ki == NCH - 1))
                    hd = pcp.tile([128, 512], F16, tag=f"hid{mi}", name=f"hid{mi}")
                    nc.scalar.activation(hd[:], h1[:], Relu,
                                         bias=fc1b_sb[:, mi:mi + 1])
                    hid.append(hd)
                orows = [pcp.tile([128, O], F32, tag=f"orow{ri}", name=f"orow{ri}")
                         for ri in range(4)]
                for oi in range(O // 128):
                    o2 = psC2.tile([128, 512], F32, tag="o2", name="o2")
                    for ki in range(NCH):
                        nc.tensor.matmul(
                            o2[:], fc2w_sb[ki][:, oi * 128:(oi + 1) * 128],
                            hid[ki][:], start=(ki == 0), stop=(ki == NCH - 1))
                    ob = pcp.tile([128, 512], F32, tag="obC", name="obC")
                    nc.scalar.activation(ob[:], o2[:], Ident,
                                         bias=fc2b_sb[:, oi:oi + 1])
                    for ri in range(4):
                        ps = psTC.tile([128, 128], F32, tag="ptC", name="ptC")
                        nc.tensor.transpose(ps[:],
                                            ob[:, ri * 128:(ri + 1) * 128],
                                            eye128fc[:])
                        nc.vector.tensor_copy(
                            orows[ri][:, oi * 128:(oi + 1) * 128], ps[:])
                for ri in range(4):
                    nc.sync.dma_start(
                        out_d[r0 + ri * 128:r0 + (ri + 1) * 128, :],
                        orows[ri][:])

    nc.compile()
    return nc


_NC_CACHE = {}


def _host_prep(reference_sample, i_cnd_series, emb, w_ih, w_hh, b_ih, b_hh,
               fc1_w, fc1_b, fc2_w, fc2_b, T):
    w_ih = np.asarray(w_ih, np.float32)
    w_hh = np.asarray(w_hh, np.float32)
    b_ih = np.asarray(b_ih, np.float32)
    b_hh = np.asarray(b_hh, np.float32)
    # rz rows (0:2H) get b_hh folded into the gx bias; n rows must have
    # b_hh = 0 (r multiplies W_n@h only -- nonzero b_hh_n unsupported)
    if np.any(np.abs(b_hh[RZW:]) > 0):
        raise NotImplementedError("nonzero b_hh n-gate not supported")
    bias_row = b_ih.copy()
    bias_row[:RZW] += b_hh[:RZW]
    # column scales: rz scaled by S_TOT (enters psum), n unscaled (DVE add)
    colscale = np.concatenate([np.full(RZW, S_TOT, np.float32),
                               np.ones(THREEH - RZW, np.float32)])
    G_tab = ((np.asarray(emb, np.float32) @ w_ih[:, :EMB].T)
             + bias_row[None, :]) * colscale[None, :]
    WcT = w_ih[:, EMB:].T.copy() * colscale[None, :]
    wstack = np.concatenate([G_tab, WcT], 0).astype(BF16)
    # W_hh^T scaled, padded 896->1024, pair layout [p*128+q, i*3H+n]
    whT = w_hh.T.copy() * S_W                      # [H, 3H]
    whT = np.concatenate([whT[:, :RZW],
                          np.zeros((H, GPAD), np.float32),
                          whT[:, RZW:]], 1)        # [H, PW]
    whT = np.concatenate([whT, np.zeros((NPAIR * 256 - H, PW),
                                        np.float32)], 0)
    whhp = (whT.reshape(NPAIR, 2, 128, PW)
            .transpose(0, 2, 1, 3).reshape(NPAIR * 128, 2 * PW)
            .astype(NP8))
    fc1wt = np.asarray(fc1_w, np.float32).T.copy().astype(NP16)
    fc2wt = np.asarray(fc2_w, np.float32).T.copy().astype(NP16)
    fc1bt = np.asarray(fc1_b, np.float32).reshape(FCD // 128, 128).T.copy()
    fc2bt = np.asarray(fc2_b, np.float32).reshape(O // 128, 128).T.copy()
    eye4h = np.eye(BL, dtype=NP16)
    eye416 = np.zeros((BL, 16), np.float32)
    eye416[:, :BL] = np.eye(BL)
    eye416 = eye416.astype(BF16)
    eye128f = np.eye(128, dtype=np.float32)
    shared = dict(wstack=wstack, whhp=whhp, fc1wt=fc1wt, fc2wt=fc2wt,
                  fc1bt=fc1bt, fc2bt=fc2bt, eye4h=eye4h, eye416=eye416,
                  eye128f=eye128f)
    sample = np.asarray(reference_sample)
    cnd = np.asarray(i_cnd_series, np.float32)
    in_maps = []
    for c in range(NCORES):
        sl = slice(c * BL, (c + 1) * BL)
        idx = sample[sl, :T].T.reshape(-1).astype(np.float32)  # (T*BL,)
        idxb = np.broadcast_to(idx[None, :], (128, T * BL)).copy()
        cndr = np.ascontiguousarray(
            cnd[sl, :T].transpose(1, 0, 2)).reshape(T * BL, C2)
        in_maps.append(dict(idxb=idxb, cndr=cndr, **shared))
    return in_maps


def kernel(reference_sample, i_cnd_series, emb, w_ih, w_hh, b_ih, b_hh,
           fc1_w, fc1_b, fc2_w, fc2_b, T=None, unroll=16, dumw=0):
    T = T or np.asarray(reference_sample).shape[1]
    in_maps = _host_prep(reference_sample, i_cnd_series, emb, w_ih, w_hh,
                         b_ih, b_hh, fc1_w, fc1_b, fc2_w, fc2_b, T)
    key = (T, unroll)
    if key not in _NC_CACHE:
        _NC_CACHE[key] = build_nc(T, unroll, dumw)
    nc = _NC_CACHE[key]
    res = run_bass_kernel_spmd(nc, in_maps, core_ids=list(range(NCORES)))
    outs = []
    for c in range(NCORES):
        o = res.results[c]["outp"].reshape(T, BL, O).transpose(1, 0, 2)
        outs.append(o)
    return np.concatenate(outs, 0).astype(np.float32)


# revision 8
# speedup vs baseline: 1.1898x; 1.1898x over previous
"""GRU GenRNN Trainium2 kernel v2 (nn_C_eAR_GenRNN).

Data-parallel over batch (B=32 -> 4 seq/core). Per core:
  Phase A: gates_x = [onehot(idx) | cnd] @ [G_tab | WcT] (PE GEMM bf16).
           rz columns scaled by S_TOT (absorbed into recurrence psum);
           n columns UNSCALED (consumed on DVE pre-tanh).
  Phase R: sequential GRU scan. W_hh@h in fp8e4m3 DoubleRow (weights
           moving, hT stationary). Post-matmul chain in fp16; hidden
           state kept gate-major [128, chunk, batch] so the h-update
           runs on all 128 lanes; small z/n tensors transposed instead
           of h. Dummy PE matmuls keep the tensor engine p-state warm.
  Phase C: hidden = relu(o_rnn @ fc1.T + b); out = hidden @ fc2.T + b.
           Reads h pre-transposed from DRAM (no transposes).
"""
import numpy as np
import ml_dtypes

import concourse.bass as bass
import concourse.bacc as bacc
import concourse.tile as tile
from concourse import mybir
from concourse.bass_utils import run_bass_kernel_spmd

BF16 = ml_dtypes.bfloat16
NP8 = ml_dtypes.float8_e4m3
NP16 = np.float16
F32 = mybir.dt.float32
F16 = mybir.dt.float16
BF = mybir.dt.bfloat16
FP8 = mybir.dt.float8e4

B, T_FULL, H, EMB, C2, O, FCD = 32, 4096, 896, 256, 512, 512, 896
NCORES, BL = 8, 4
THREEH = 3 * H
NCH = H // 128   # 7 chunks of 128 hidden units
NPAIR = 4        # fp8 DoubleRow pairs: H padded 896 -> 1024 = 4 x 256
RZW = 2 * H      # 1792 (r|z block)
GPAD = 256       # psum pad so the n block starts bank-aligned at 2048
PW = THREEH + GPAD  # 2944: psum/whhp column layout [rz | pad | n]

S_W = 256.0      # W_hh scale into fp8e4m3 normal range
S_H = 64.0       # h scale into fp8e4m3 normal range
S_TOT = S_W * S_H
INV_S = 1.0 / S_TOT

Sig = mybir.ActivationFunctionType.Sigmoid
Tanh = mybir.ActivationFunctionType.Tanh
Relu = mybir.ActivationFunctionType.Relu
Ident = mybir.ActivationFunctionType.Identity
DR = mybir.MatmulPerfMode.DoubleRow


def _mm_windows(lo, hi):
    """Bank-aligned (<=512, non-straddling) windows covering [lo, hi)."""
    out = []
    pos = lo
    while pos < hi:
        end = min(hi, (pos // 512 + 1) * 512)
        out.append((pos, end - pos))
        pos = end
    return out


W_RZ = _mm_windows(0, RZW)          # (0,512)(512,512)(1024,512)(1536,256)
W_N = _mm_windows(RZW + GPAD, PW)   # (2048,512)(2560,384)
W_ALL = W_RZ + W_N


def build_nc(T=T_FULL, unroll=8, dumw=0):
    RT = T * BL
    nc = bacc.Bacc("TRN2", target_bir_lowering=False, debug=False,
                   num_devices=NCORES)
    ein = lambda n, s, d: nc.dram_tensor(n, s, d, kind="ExternalInput").ap()
    idxb_d = ein("idxb", [128, RT], F32)
    cndr_d = ein("cndr", [RT, C2], F32)
    wstack_d = ein("wstack", [1024, THREEH], BF)
    whhp_d = ein("whhp", [NPAIR * 128, 2 * PW], FP8)
    fc1wt_d = ein("fc1wt", [H, FCD], F16)
    fc2wt_d = ein("fc2wt", [FCD, O], F16)
    fc1bt_d = ein("fc1bt", [128, FCD // 128], F32)
    fc2bt_d = ein("fc2bt", [128, O // 128], F32)
    eye4h_d = ein("eye4h", [BL, BL], F16)
    eye416_d = ein("eye416", [BL, 16], BF)
    eye128f_d = ein("eye128f", [128, 128], F32)
    gx_d = nc.dram_tensor("gx_i", [RT, THREEH], BF).ap()
    BLK = BL * unroll
    hT_d = nc.dram_tensor("hT_i", [128, NCH, RT], F16).ap()
    out_d = nc.dram_tensor("outp", [RT, O], F32, kind="ExternalOutput").ap()

    with tile.TileContext(nc) as tc:
        # ---------------- Phase A: gates_x GEMM ----------------
        with (
            tc.tile_pool(name="wA", bufs=1) as wA,
            tc.tile_pool(name="pa", bufs=3) as pa,
            tc.tile_pool(name="cstA", bufs=1) as cstA,
            tc.tile_pool(name="psA", bufs=6, space="PSUM") as psA,
            tc.tile_pool(name="psTA", bufs=2, space="PSUM") as psTA,
        ):
            eye128f = cstA.tile([128, 128], F32)
            nc.sync.dma_start(eye128f[:], eye128f_d[:])
            wstack_sb = []
            for k in range(8):
                wt = wA.tile([128, THREEH], BF, tag=f"wst{k}", name=f"wst{k}")
                nc.sync.dma_start(wt[:], wstack_d[k * 128:(k + 1) * 128, :])
                wstack_sb.append(wt)
            it32 = cstA.tile([128, 1], mybir.dt.int32)
            nc.gpsimd.iota(it32[:], pattern=[[1, 1]], base=0, channel_multiplier=1)
            ocs = []
            for k in range(4):
                oc = cstA.tile([128, 1], F32, tag=f"oc{k}", name=f"oc{k}")
                nc.vector.tensor_scalar_add(oc[:], it32[:], float(128 * k))
                ocs.append(oc)

            for rt in range(RT // 128):
                idxt = pa.tile([128, 128], F32)
                nc.sync.dma_start(idxt[:], idxb_d[:, rt * 128:(rt + 1) * 128])
                xT = []
                for k in range(4):
                    oh = pa.tile([128, 128], BF, tag=f"oh{k}", name=f"oh{k}")
                    nc.vector.tensor_scalar(oh[:], idxt[:], ocs[k][:], None,
                                            op0=mybir.AluOpType.is_equal)
                    xT.append(oh)
                for k in range(4):
                    ct = pa.tile([128, 128], F32, tag=f"ct{k}", name=f"ct{k}")
                    nc.sync.dma_start(
                        ct[:], cndr_d[rt * 128:(rt + 1) * 128,
                                      k * 128:(k + 1) * 128])
                    pst = psTA.tile([128, 128], F32)
                    nc.tensor.transpose(pst[:], ct[:], eye128f[:])
                    cb = pa.tile([128, 128], BF, tag=f"cb{k}", name=f"cb{k}")
                    nc.vector.tensor_copy(cb[:], pst[:])
                    xT.append(cb)
                gxall = pa.tile([128, THREEH], BF, tag="gxall", name="gxall")
                for (w0, wl) in _mm_windows(0, THREEH):
                    pg = psA.tile([128, 512], F32, tag="pgA", name="pgA")
                    for k in range(8):
                        nc.tensor.matmul(pg[:, :wl], xT[k][:],
                                         wstack_sb[k][:, w0:w0 + wl],
                                         start=(k == 0), stop=(k == 7))
                    nc.vector.tensor_copy(gxall[:, w0:w0 + wl], pg[:, :wl])
                nc.sync.dma_start(gx_d[rt * 128:(rt + 1) * 128, :], gxall[:])

        # ---------------- Phase R: GRU recurrence ----------------
        with (
            tc.tile_pool(name="wR", bufs=1) as wR,
            tc.tile_pool(name="stR", bufs=1) as stR,
            tc.tile_pool(name="pr", bufs=3) as pr,
            tc.tile_pool(name="prg", bufs=4) as prg,
            tc.tile_pool(name="prr", bufs=2) as prr,
            tc.tile_pool(name="psR", bufs=1, space="PSUM") as psR,
            tc.tile_pool(name="psT", bufs=1, space="PSUM") as psT,
            tc.tile_pool(name="psD", bufs=1, space="PSUM") as psD,
        ):
            whhp_sb = []
            for p in range(NPAIR):
                wt = wR.tile([128, 2, PW], FP8, tag=f"whp{p}", name=f"whp{p}")
                nc.sync.dma_start(wt[:], whhp_d[p * 128:(p + 1) * 128, :])
                whhp_sb.append(wt)
            eye4h = wR.tile([BL, BL], F16, tag="eye4h")
            nc.sync.dma_start(eye4h[:], eye4h_d[:])
            eye416 = wR.tile([BL, 16], BF, tag="eye416")
            nc.sync.dma_start(eye416[:], eye416_d[:])
            dum8 = wR.tile([128, 2, 16], FP8, tag="dum8")
            nc.vector.memset(dum8[:], 0.0)
            # gate-major h state: hcarry [128, 8, 4] fp16 (chunk 7 = pad, 0)
            hcarry = stR.tile([128, NPAIR * 2, BL], F16, tag="hcarry")
            nc.vector.memset(hcarry[:], 0.0)
            # fp8 stationary for the DR matmul: [128, pair, sub, 16]
            hT8 = stR.tile([128, NPAIR, 2, 16], FP8, tag="hT8")
            nc.vector.memset(hT8[:], 0.0)
            # z / pre-tanh n transposed into psum: [128, {z,nn}, chunk(8), b]
            # (fp16 psum memset is illegal; zero pad chunk 7 via transposes)
            ptr = psT.tile([128, 2, NPAIR * 2, BL], F16, tag="ptr")
            zpad = stR.tile([BL, 128], F16, tag="zpad")
            nc.vector.memset(zpad[:], 0.0)
            for zn in range(2):
                nc.tensor.transpose(ptr[:, zn, NCH, :], zpad[:], eye4h[:])

            def step(row2, ring, u, prev):
                # gx for two steps per DMA, packed side-by-side in free dim
                if u % 2 == 0:
                    gxp = prg.tile([BL, 2, THREEH], BF, tag="gxp", name="gxp")
                    nc.sync.dma_start(
                        gxp[:],
                        gx_d[row2, :].rearrange("(s p) d -> p s d", s=2))
                    step.gxp = gxp
                gxb = step.gxp[:, u % 2, :]
                pg = psR.tile([16, PW], F32, tag="pg", name="pg")
                # dummy matmuls: independent PE work to keep p-state warm
                for d in range(dumw):
                    pd = psD.tile([16, 512], F32, tag="pd", name="pd")
                    nc.tensor.matmul(pd[:], dum8[:],
                                     whhp_sb[d % NPAIR][:, :, 0:512],
                                     start=True, stop=True, perf_mode=DR)
                # gx(rz) into psum first (independent of hT8)
                for (w0, wl) in W_RZ:
                    nc.tensor.matmul(pg[:, w0:w0 + wl], eye416[:],
                                     gxb[:, w0:w0 + wl],
                                     start=True, stop=False)
                # DR passes: weights moving, hT8 stationary
                for p in range(NPAIR):
                    for (w0, wl) in W_ALL:
                        nc.tensor.matmul(
                            pg[:, w0:w0 + wl], hT8[:, p, :, :],
                            whhp_sb[p][:, :, w0:w0 + wl],
                            start=(p == 0 and w0 >= RZW),
                            stop=(p == NPAIR - 1), perf_mode=DR)
                # sigmoid(rz) -> fp16 batch-major
                rz16 = pr.tile([BL, RZW], F16, tag="rz16", name="rz16")
                nc.scalar.activation(rz16[:], pg[0:BL, 0:RZW], Sig,
                                     scale=INV_S)
                # raw hn -> fp16 (parallel on DVE)
                hn16 = pr.tile([BL, H], F16, tag="hn16", name="hn16")
                nc.vector.tensor_scalar_mul(hn16[:], pg[0:BL, RZW + GPAD:PW],
                                            INV_S)
                rnt = pr.tile([BL, H], F16, tag="rnt", name="rnt")
                nc.vector.tensor_mul(rnt[:], rz16[:, 0:H], hn16[:])
                nnt = pr.tile([BL, H], F16, tag="nnt", name="nnt")
                nc.vector.tensor_add(nnt[:], rnt[:], gxb[:, RZW:THREEH])
                # transpose z and pre-tanh n into ptr (batch -> gate major)
                for c in range(NCH):
                    nc.tensor.transpose(ptr[:, 0, c, :],
                                        rz16[:, H + c * 128:H + (c + 1) * 128],
                                        eye4h[:])
                for c in range(NCH):
                    nc.tensor.transpose(ptr[:, 1, c, :],
                                        nnt[:, c * 128:(c + 1) * 128],
                                        eye4h[:])
                # gate-major tanh on all 128 lanes
                nbT = pr.tile([128, NPAIR * 2, BL], F16, tag="nbT", name="nbT")
                nc.scalar.activation(nbT[:], ptr[:, 1, :, :], Tanh)
                # gate-major h update: h' = n + z*(h - n)
                dhn = pr.tile([128, NPAIR * 2, BL], F16, tag="dhn", name="dhn")
                nc.vector.tensor_sub(dhn[:], prev[:], nbT[:])
                zd = pr.tile([128, NPAIR * 2, BL], F16, tag="zd", name="zd")
                nc.vector.tensor_mul(zd[:], ptr[:, 0, :, :], dhn[:])
                cur = ring[:, :, u, :]
                nc.vector.tensor_add(cur, nbT[:], zd[:])
                # fp8 stationary for next step
                nc.vector.tensor_scalar_mul(
                    hT8[:, :, :, 0:BL],
                    cur.rearrange("p (a b) c -> p a b c", b=2), S_H)
                return cur

            with tc.For_i(0, RT, BLK) as ivr:
                ring = prr.tile([128, NPAIR * 2, unroll, BL], F16,
                                tag="ring", name="ring")
                prev = hcarry
                for u in range(unroll):
                    prev = step(bass.ds(ivr + BL * (u - u % 2), 2 * BL),
                                ring, u, prev)
                nc.vector.tensor_copy(hcarry[:], prev)
                # contiguous h-block store per chunk (64B/partition runs)
                for c in range(NCH):
                    nc.sync.dma_start(
                        hT_d[:, c, bass.ds(ivr, BLK)],
                        ring[:, c, :, :].rearrange("p u b -> p (u b)"))

        # ---------------- Phase C: FC layers ----------------
        with (
            tc.tile_pool(name="wC", bufs=1) as wC,
            tc.tile_pool(name="pcp", bufs=2) as pcp,
            tc.tile_pool(name="psC1", bufs=2, space="PSUM") as psC1,
            tc.tile_pool(name="psC2", bufs=2, space="PSUM") as psC2,
            tc.tile_pool(name="psTC", bufs=2, space="PSUM") as psTC,
        ):
            eye128fc = wC.tile([128, 128], F32, tag="eye128fc")
            nc.sync.dma_start(eye128fc[:], eye128f_d[:])
            fc1w_sb, fc2w_sb = [], []
            for k in range(NCH):
                wt = wC.tile([128, FCD], F16, tag=f"fc1w{k}", name=f"fc1w{k}")
                nc.sync.dma_start(wt[:], fc1wt_d[k * 128:(k + 1) * 128, :])
                fc1w_sb.append(wt)
                wt2 = wC.tile([128, O], F16, tag=f"fc2w{k}", name=f"fc2w{k}")
                nc.sync.dma_start(wt2[:], fc2wt_d[k * 128:(k + 1) * 128, :])
                fc2w_sb.append(wt2)
            fc1b_sb = wC.tile([128, FCD // 128], F32, tag="fc1b")
            nc.sync.dma_start(fc1b_sb[:], fc1bt_d[:])
            fc2b_sb = wC.tile([128, O // 128], F32, tag="fc2b")
            nc.sync.dma_start(fc2b_sb[:], fc2bt_d[:])

            n_rc = RT // 512
            for rc in range(n_rc):
                r0 = rc * 512
                oT = []
                for fi in range(NCH):
                    ot = pcp.tile([128, 512], F16, tag=f"oT{fi}", name=f"oT{fi}")
                    nc.sync.dma_start(ot[:], hT_d[:, fi, r0:r0 + 512])
                    oT.append(ot)
                hid = []
                for mi in range(NCH):
                    h1 = psC1.tile([128, 512], F32, tag="h1", name="h1")
                    for ki in range(NCH):
                        nc.tensor.matmul(
                            h1[:], fc1w_sb[ki][:, mi * 128:(mi + 1) * 128],
                            oT[ki][:], start=(ki == 0), stop=(ki == NCH - 1))
                    hd = pcp.tile([128, 512], F16, tag=f"hid{mi}", name=f"hid{mi}")
                    nc.scalar.activation(hd[:], h1[:], Relu,
                                         bias=fc1b_sb[:, mi:mi + 1])
                    hid.append(hd)
                orows = [pcp.tile([128, O], F32, tag=f"orow{ri}", name=f"orow{ri}")
                         for ri in range(4)]
                for oi in range(O // 128):
                    o2 = psC2.tile([128, 512], F32, tag="o2", name="o2")
                    for ki in range(NCH):
                        nc.tensor.matmul(
                            o2[:], fc2w_sb[ki][:, oi * 128:(oi + 1) * 128],
                            hid[ki][:], start=(ki == 0), stop=(ki == NCH - 1))
                    ob = pcp.tile([128, 512], F32, tag="obC", name="obC")
                    nc.scalar.activation(ob[:], o2[:], Ident,
                                         bias=fc2b_sb[:, oi:oi + 1])
                    for ri in range(4):
                        ps = psTC.tile([128, 128], F32, tag="ptC", name="ptC")
                        nc.tensor.transpose(ps[:],
                                            ob[:, ri * 128:(ri + 1) * 128],
                                            eye128fc[:])
                        nc.vector.tensor_copy(
                            orows[ri][:, oi * 128:(oi + 1) * 128], ps[:])
                for ri in range(4):
                    nc.sync.dma_start(
                        out_d[r0 + ri * 128:r0 + (ri + 1) * 128, :],
                        orows[ri][:])

    nc.compile()
    return nc


_NC_CACHE = {}


def _host_prep(reference_sample, i_cnd_series, emb, w_ih, w_hh, b_ih, b_hh,
               fc1_w, fc1_b, fc2_w, fc2_b, T):
    w_ih = np.asarray(w_ih, np.float32)
    w_hh = np.asarray(w_hh, np.float32)
    b_ih = np.asarray(b_ih, np.float32)
    b_hh = np.asarray(b_hh, np.float32)
    # rz rows (0:2H) get b_hh folded into the gx bias; n rows must have
    # b_hh = 0 (r multiplies W_n@h only -- nonzero b_hh_n unsupported)
    if np.any(np.abs(b_hh[RZW:]) > 0):
        raise NotImplementedError("nonzero b_hh n-gate not supported")
    bias_row = b_ih.copy()
    bias_row[:RZW] += b_hh[:RZW]
    # column scales: rz scaled by S_TOT (enters psum), n unscaled (DVE add)
    colscale = np.concatenate([np.full(RZW, S_TOT, np.float32),
                               np.ones(THREEH - RZW, np.float32)])
    G_tab = ((np.asarray(emb, np.float32) @ w_ih[:, :EMB].T)
             + bias_row[None, :]) * colscale[None, :]
    WcT = w_ih[:, EMB:].T.copy() * colscale[None, :]
    wstack = np.concatenate([G_tab, WcT], 0).astype(BF16)
    # W_hh^T scaled, padded 896->1024, pair layout [p*128+q, i*3H+n]
    whT = w_hh.T.copy() * S_W                      # [H, 3H]
    whT = np.concatenate([whT[:, :RZW],
                          np.zeros((H, GPAD), np.float32),
                          whT[:, RZW:]], 1)        # [H, PW]
    whT = np.concatenate([whT, np.zeros((NPAIR * 256 - H, PW),
                                        np.float32)], 0)
    whhp = (whT.reshape(NPAIR, 2, 128, PW)
            .transpose(0, 2, 1, 3).reshape(NPAIR * 128, 2 * PW)
            .astype(NP8))
    fc1wt = np.asarray(fc1_w, np.float32).T.copy().astype(NP16)
    fc2wt = np.asarray(fc2_w, np.float32).T.copy().astype(NP16)
    fc1bt = np.asarray(fc1_b, np.float32).reshape(FCD // 128, 128).T.copy()
    fc2bt = np.asarray(fc2_b, np.float32).reshape(O // 128, 128).T.copy()
    eye4h = np.eye(BL, dtype=NP16)
    eye416 = np.zeros((BL, 16), np.float32)
    eye416[:, :BL] = np.eye(BL)
    eye416 = eye416.astype(BF16)
    eye128f = np.eye(128, dtype=np.float32)
    shared = dict(wstack=wstack, whhp=whhp, fc1wt=fc1wt, fc2wt=fc2wt,
                  fc1bt=fc1bt, fc2bt=fc2bt, eye4h=eye4h, eye416=eye416,
                  eye128f=eye128f)
    sample = np.asarray(reference_sample)
    cnd = np.asarray(i_cnd_series, np.float32)
    in_maps = []
    for c in range(NCORES):
        sl = slice(c * BL, (c + 1) * BL)
        idx = sample[sl, :T].T.reshape(-1).astype(np.float32)  # (T*BL,)
        idxb = np.broadcast_to(idx[None, :], (128, T * BL)).copy()
        cndr = np.ascontiguousarray(
            cnd[sl, :T].transpose(1, 0, 2)).reshape(T * BL, C2)
        in_maps.append(dict(idxb=idxb, cndr=cndr, **shared))
    return in_maps


def kernel(reference_sample, i_cnd_series, emb, w_ih, w_hh, b_ih, b_hh,
           fc1_w, fc1_b, fc2_w, fc2_b, T=None, unroll=16, dumw=0):
    T = T or np.asarray(reference_sample).shape[1]
    in_maps = _host_prep(reference_sample, i_cnd_series, emb, w_ih, w_hh,
                         b_ih, b_hh, fc1_w, fc1_b, fc2_w, fc2_b, T)
    key = (T, unroll)
    if key not in _NC_CACHE:
        _NC_CACHE[key] = build_nc(T, unroll, dumw)
    nc = _NC_CACHE[key]
    res = run_bass_kernel_spmd(nc, in_maps, core_ids=list(range(NCORES)))
    outs = []
    for c in range(NCORES):
        o = res.results[c]["outp"].reshape(T, BL, O).transpose(1, 0, 2)
        outs.append(o)
    return np.concatenate(outs, 0).astype(np.float32)


# revision 9
# speedup vs baseline: 1.1903x; 1.0005x over previous
"""GRU GenRNN Trainium2 kernel v2 (nn_C_eAR_GenRNN).

Data-parallel over batch (B=32 -> 4 seq/core). Per core:
  Phase A: gates_x = [onehot(idx) | cnd] @ [G_tab | WcT] (PE GEMM bf16).
           rz columns scaled by S_TOT (absorbed into recurrence psum);
           n columns UNSCALED (consumed on DVE pre-tanh).
  Phase R: sequential GRU scan. W_hh@h in fp8e4m3 DoubleRow (weights
           moving, hT stationary). Post-matmul chain in fp16; hidden
           state kept gate-major [128, chunk, batch] so the h-update
           runs on all 128 lanes; small z/n tensors transposed instead
           of h. Dummy PE matmuls keep the tensor engine p-state warm.
  Phase C: hidden = relu(o_rnn @ fc1.T + b); out = hidden @ fc2.T + b.
           Reads h pre-transposed from DRAM (no transposes).
"""
import numpy as np
import ml_dtypes

import concourse.bass as bass
import concourse.bacc as bacc
import concourse.tile as tile
from concourse import mybir
from concourse.bass_utils import run_bass_kernel_spmd

BF16 = ml_dtypes.bfloat16
NP8 = ml_dtypes.float8_e4m3
NP16 = np.float16
F32 = mybir.dt.float32
F16 = mybir.dt.float16
BF = mybir.dt.bfloat16
FP8 = mybir.dt.float8e4

B, T_FULL, H, EMB, C2, O, FCD = 32, 4096, 896, 256, 512, 512, 896
NCORES, BL = 8, 4
THREEH = 3 * H
NCH = H // 128   # 7 chunks of 128 hidden units
NPAIR = 4        # fp8 DoubleRow pairs: H padded 896 -> 1024 = 4 x 256
RZW = 2 * H      # 1792 (r|z block)
GPAD = 256       # psum pad so the n block starts bank-aligned at 2048
PW = THREEH + GPAD  # 2944: psum/whhp column layout [rz | pad | n]

S_W = 256.0      # W_hh scale into fp8e4m3 normal range
S_H = 64.0       # h scale into fp8e4m3 normal range
S_TOT = S_W * S_H
INV_S = 1.0 / S_TOT

Sig = mybir.ActivationFunctionType.Sigmoid
Tanh = mybir.ActivationFunctionType.Tanh
Relu = mybir.ActivationFunctionType.Relu
Ident = mybir.ActivationFunctionType.Identity
DR = mybir.MatmulPerfMode.DoubleRow


def _mm_windows(lo, hi):
    """Bank-aligned (<=512, non-straddling) windows covering [lo, hi)."""
    out = []
    pos = lo
    while pos < hi:
        end = min(hi, (pos // 512 + 1) * 512)
        out.append((pos, end - pos))
        pos = end
    return out


W_RZ = _mm_windows(0, RZW)          # (0,512)(512,512)(1024,512)(1536,256)
W_N = _mm_windows(RZW + GPAD, PW)   # (2048,512)(2560,384)
W_ALL = W_RZ + W_N


def build_nc(T=T_FULL, unroll=8, dumw=0):
    RT = T * BL
    nc = bacc.Bacc("TRN2", target_bir_lowering=False, debug=False,
                   num_devices=NCORES)
    ein = lambda n, s, d: nc.dram_tensor(n, s, d, kind="ExternalInput").ap()
    idxb_d = ein("idxb", [128, RT], F32)
    cndr_d = ein("cndr", [RT, C2], F32)
    wstack_d = ein("wstack", [1024, THREEH], BF)
    whhp_d = ein("whhp", [NPAIR * 128, 2 * PW], FP8)
    fc1wt_d = ein("fc1wt", [H, FCD], F16)
    fc2wt_d = ein("fc2wt", [FCD, O], F16)
    fc1bt_d = ein("fc1bt", [128, FCD // 128], F32)
    fc2bt_d = ein("fc2bt", [128, O // 128], F32)
    eye4h_d = ein("eye4h", [BL, BL], F16)
    eye416_d = ein("eye416", [BL, 16], BF)
    eye128f_d = ein("eye128f", [128, 128], F32)
    gx_d = nc.dram_tensor("gx_i", [RT, THREEH], BF).ap()
    BLK = BL * unroll
    hT_d = nc.dram_tensor("hT_i", [128, NCH, RT], F16).ap()
    out_d = nc.dram_tensor("outp", [RT, O], F32, kind="ExternalOutput").ap()

    with tile.TileContext(nc) as tc:
        # ---------------- Phase A: gates_x GEMM ----------------
        with (
            tc.tile_pool(name="wA", bufs=1) as wA,
            tc.tile_pool(name="pa", bufs=3) as pa,
            tc.tile_pool(name="cstA", bufs=1) as cstA,
            tc.tile_pool(name="psA", bufs=6, space="PSUM") as psA,
            tc.tile_pool(name="psTA", bufs=2, space="PSUM") as psTA,
        ):
            eye128f = cstA.tile([128, 128], F32)
            nc.sync.dma_start(eye128f[:], eye128f_d[:])
            wstack_sb = []
            for k in range(8):
                wt = wA.tile([128, THREEH], BF, tag=f"wst{k}", name=f"wst{k}")
                nc.sync.dma_start(wt[:], wstack_d[k * 128:(k + 1) * 128, :])
                wstack_sb.append(wt)
            it32 = cstA.tile([128, 1], mybir.dt.int32)
            nc.gpsimd.iota(it32[:], pattern=[[1, 1]], base=0, channel_multiplier=1)
            ocs = []
            for k in range(4):
                oc = cstA.tile([128, 1], F32, tag=f"oc{k}", name=f"oc{k}")
                nc.vector.tensor_scalar_add(oc[:], it32[:], float(128 * k))
                ocs.append(oc)

            for rt in range(RT // 128):
                idxt = pa.tile([128, 128], F32)
                nc.sync.dma_start(idxt[:], idxb_d[:, rt * 128:(rt + 1) * 128])
                xT = []
                for k in range(4):
                    oh = pa.tile([128, 128], BF, tag=f"oh{k}", name=f"oh{k}")
                    nc.vector.tensor_scalar(oh[:], idxt[:], ocs[k][:], None,
                                            op0=mybir.AluOpType.is_equal)
                    xT.append(oh)
                for k in range(4):
                    ct = pa.tile([128, 128], F32, tag=f"ct{k}", name=f"ct{k}")
                    nc.sync.dma_start(
                        ct[:], cndr_d[rt * 128:(rt + 1) * 128,
                                      k * 128:(k + 1) * 128])
                    pst = psTA.tile([128, 128], F32)
                    nc.tensor.transpose(pst[:], ct[:], eye128f[:])
                    cb = pa.tile([128, 128], BF, tag=f"cb{k}", name=f"cb{k}")
                    nc.vector.tensor_copy(cb[:], pst[:])
                    xT.append(cb)
                gxall = pa.tile([128, THREEH], BF, tag="gxall", name="gxall")
                for (w0, wl) in _mm_windows(0, THREEH):
                    pg = psA.tile([128, 512], F32, tag="pgA", name="pgA")
                    for k in range(8):
                        nc.tensor.matmul(pg[:, :wl], xT[k][:],
                                         wstack_sb[k][:, w0:w0 + wl],
                                         start=(k == 0), stop=(k == 7))
                    nc.vector.tensor_copy(gxall[:, w0:w0 + wl], pg[:, :wl])
                nc.sync.dma_start(gx_d[rt * 128:(rt + 1) * 128, :], gxall[:])

        # ---------------- Phase R: GRU recurrence ----------------
        with (
            tc.tile_pool(name="wR", bufs=1) as wR,
            tc.tile_pool(name="stR", bufs=1) as stR,
            tc.tile_pool(name="pr", bufs=3) as pr,
            tc.tile_pool(name="prg", bufs=4) as prg,
            tc.tile_pool(name="prr", bufs=2) as prr,
            tc.tile_pool(name="psR", bufs=1, space="PSUM") as psR,
            tc.tile_pool(name="psT", bufs=1, space="PSUM") as psT,
            tc.tile_pool(name="psD", bufs=1, space="PSUM") as psD,
        ):
            whhp_sb = []
            for p in range(NPAIR):
                wt = wR.tile([128, 2, PW], FP8, tag=f"whp{p}", name=f"whp{p}")
                nc.sync.dma_start(wt[:], whhp_d[p * 128:(p + 1) * 128, :])
                whhp_sb.append(wt)
            eye4h = wR.tile([BL, BL], F16, tag="eye4h")
            nc.sync.dma_start(eye4h[:], eye4h_d[:])
            eye416 = wR.tile([BL, 16], BF, tag="eye416")
            nc.sync.dma_start(eye416[:], eye416_d[:])
            dum8 = wR.tile([128, 2, 16], FP8, tag="dum8")
            nc.vector.memset(dum8[:], 0.0)
            # gate-major h state: hcarry [128, 8, 4] fp16 (chunk 7 = pad, 0)
            hcarry = stR.tile([128, NPAIR * 2, BL], F16, tag="hcarry")
            nc.vector.memset(hcarry[:], 0.0)
            # fp8 stationary for the DR matmul: [128, pair, sub, 16]
            hT8 = stR.tile([128, NPAIR, 2, 16], FP8, tag="hT8")
            nc.vector.memset(hT8[:], 0.0)
            # z / pre-tanh n transposed into psum: [128, {z,nn}, chunk(8), b]
            # (fp16 psum memset is illegal; zero pad chunk 7 via transposes)
            ptr = psT.tile([128, 2, NPAIR * 2, BL], F16, tag="ptr")
            zpad = stR.tile([BL, 128], F16, tag="zpad")
            nc.vector.memset(zpad[:], 0.0)
            for zn in range(2):
                nc.tensor.transpose(ptr[:, zn, NCH, :], zpad[:], eye4h[:])

            def step(row2, ring, u, prev):
                # gx for four steps per DMA, packed side-by-side in free dim
                if u % 4 == 0:
                    gxp = prg.tile([BL, 4, THREEH], BF, tag="gxp", name="gxp")
                    nc.sync.dma_start(
                        gxp[:],
                        gx_d[row2, :].rearrange("(s p) d -> p s d", s=4))
                    step.gxp = gxp
                gxb = step.gxp[:, u % 4, :]
                pg = psR.tile([16, PW], F32, tag="pg", name="pg")
                # dummy matmuls: independent PE work to keep p-state warm
                for d in range(dumw):
                    pd = psD.tile([16, 512], F32, tag="pd", name="pd")
                    nc.tensor.matmul(pd[:], dum8[:],
                                     whhp_sb[d % NPAIR][:, :, 0:512],
                                     start=True, stop=True, perf_mode=DR)
                # gx(rz) into psum first (independent of hT8)
                for (w0, wl) in W_RZ:
                    nc.tensor.matmul(pg[:, w0:w0 + wl], eye416[:],
                                     gxb[:, w0:w0 + wl],
                                     start=True, stop=False)
                # DR passes: weights moving, hT8 stationary
                for p in range(NPAIR):
                    for (w0, wl) in W_ALL:
                        nc.tensor.matmul(
                            pg[:, w0:w0 + wl], hT8[:, p, :, :],
                            whhp_sb[p][:, :, w0:w0 + wl],
                            start=(p == 0 and w0 >= RZW),
                            stop=(p == NPAIR - 1), perf_mode=DR)
                # sigmoid(rz) -> fp16 batch-major
                rz16 = pr.tile([BL, RZW], F16, tag="rz16", name="rz16")
                nc.scalar.activation(rz16[:], pg[0:BL, 0:RZW], Sig,
                                     scale=INV_S)
                # raw hn -> fp16 (parallel on DVE)
                hn16 = pr.tile([BL, H], F16, tag="hn16", name="hn16")
                nc.vector.tensor_scalar_mul(hn16[:], pg[0:BL, RZW + GPAD:PW],
                                            INV_S)
                rnt = pr.tile([BL, H], F16, tag="rnt", name="rnt")
                nc.vector.tensor_mul(rnt[:], rz16[:, 0:H], hn16[:])
                nnt = pr.tile([BL, H], F16, tag="nnt", name="nnt")
                nc.vector.tensor_add(nnt[:], rnt[:], gxb[:, RZW:THREEH])
                # transpose z and pre-tanh n into ptr (batch -> gate major)
                for c in range(NCH):
                    nc.tensor.transpose(ptr[:, 0, c, :],
                                        rz16[:, H + c * 128:H + (c + 1) * 128],
                                        eye4h[:])
                for c in range(NCH):
                    nc.tensor.transpose(ptr[:, 1, c, :],
                                        nnt[:, c * 128:(c + 1) * 128],
                                        eye4h[:])
                # gate-major tanh on all 128 lanes
                nbT = pr.tile([128, NPAIR * 2, BL], F16, tag="nbT", name="nbT")
                nc.scalar.activation(nbT[:], ptr[:, 1, :, :], Tanh)
                # gate-major h update: h' = n + z*(h - n)
                dhn = pr.tile([128, NPAIR * 2, BL], F16, tag="dhn", name="dhn")
                nc.vector.tensor_sub(dhn[:], prev[:], nbT[:])
                zd = pr.tile([128, NPAIR * 2, BL], F16, tag="zd", name="zd")
                nc.vector.tensor_mul(zd[:], ptr[:, 0, :, :], dhn[:])
                cur = ring[:, :, u, :]
                nc.vector.tensor_add(cur, nbT[:], zd[:])
                # fp8 stationary for next step
                nc.vector.tensor_scalar_mul(
                    hT8[:, :, :, 0:BL],
                    cur.rearrange("p (a b) c -> p a b c", b=2), S_H)
                return cur

            with tc.For_i(0, RT, BLK) as ivr:
                ring = prr.tile([128, NPAIR * 2, unroll, BL], F16,
                                tag="ring", name="ring")
                prev = hcarry
                for u in range(unroll):
                    prev = step(bass.ds(ivr + BL * (u - u % 4), 4 * BL),
                                ring, u, prev)
                nc.vector.tensor_copy(hcarry[:], prev)
                # contiguous h-block store per chunk (64B/partition runs)
                for c in range(NCH):
                    nc.sync.dma_start(
                        hT_d[:, c, bass.ds(ivr, BLK)],
                        ring[:, c, :, :].rearrange("p u b -> p (u b)"))

        # ---------------- Phase C: FC layers ----------------
        with (
            tc.tile_pool(name="wC", bufs=1) as wC,
            tc.tile_pool(name="pcp", bufs=2) as pcp,
            tc.tile_pool(name="psC1", bufs=2, space="PSUM") as psC1,
            tc.tile_pool(name="psC2", bufs=2, space="PSUM") as psC2,
            tc.tile_pool(name="psTC", bufs=2, space="PSUM") as psTC,
        ):
            eye128fc = wC.tile([128, 128], F32, tag="eye128fc")
            nc.sync.dma_start(eye128fc[:], eye128f_d[:])
            fc1w_sb, fc2w_sb = [], []
            for k in range(NCH):
                wt = wC.tile([128, FCD], F16, tag=f"fc1w{k}", name=f"fc1w{k}")
                nc.sync.dma_start(wt[:], fc1wt_d[k * 128:(k + 1) * 128, :])
                fc1w_sb.append(wt)
                wt2 = wC.tile([128, O], F16, tag=f"fc2w{k}", name=f"fc2w{k}")
                nc.sync.dma_start(wt2[:], fc2wt_d[k * 128:(k + 1) * 128, :])
                fc2w_sb.append(wt2)
            fc1b_sb = wC.tile([128, FCD // 128], F32, tag="fc1b")
            nc.sync.dma_start(fc1b_sb[:], fc1bt_d[:])
            fc2b_sb = wC.tile([128, O // 128], F32, tag="fc2b")
            nc.sync.dma_start(fc2b_sb[:], fc2bt_d[:])

            n_rc = RT // 512
            for rc in range(n_rc):
                r0 = rc * 512
                oT = []
                for fi in range(NCH):
                    ot = pcp.tile([128, 512], F16, tag=f"oT{fi}", name=f"oT{fi}")
                    nc.sync.dma_start(ot[:], hT_d[:, fi, r0:r0 + 512])
                    oT.append(ot)
                hid = []
                for mi in range(NCH):
                    h1 = psC1.tile([128, 512], F32, tag="h1", name="h1")
                    for ki in range(NCH):
                        nc.tensor.matmul(
                            h1[:], fc1w_sb[ki][:, mi * 128:(mi + 1) * 128],
                            oT[ki][:], start=(ki == 0), stop=(ki == NCH - 1))
                    hd = pcp.tile([128, 512], F16, tag=f"hid{mi}", name=f"hid{mi}")
                    nc.scalar.activation(hd[:], h1[:], Relu,
                                         bias=fc1b_sb[:, mi:mi + 1])
                    hid.append(hd)
                orows = [pcp.tile([128, O], F32, tag=f"orow{ri}", name=f"orow{ri}")
                         for ri in range(4)]
                for oi in range(O // 128):
                    o2 = psC2.tile([128, 512], F32, tag="o2", name="o2")
                    for ki in range(NCH):
                        nc.tensor.matmul(
                            o2[:], fc2w_sb[ki][:, oi * 128:(oi + 1) * 128],
                            hid[ki][:], start=(ki == 0), stop=(ki == NCH - 1))
                    ob = pcp.tile([128, 512], F32, tag="obC", name="obC")
                    nc.scalar.activation(ob[:], o2[:], Ident,
                                         bias=fc2b_sb[:, oi:oi + 1])
                    for ri in range(4):
                        ps = psTC.tile([128, 128], F32, tag="ptC", name="ptC")
                        nc.tensor.transpose(ps[:],
                                            ob[:, ri * 128:(ri + 1) * 128],
                                            eye128fc[:])
                        nc.vector.tensor_copy(
                            orows[ri][:, oi * 128:(oi + 1) * 128], ps[:])
                for ri in range(4):
                    nc.sync.dma_start(
                        out_d[r0 + ri * 128:r0 + (ri + 1) * 128, :],
                        orows[ri][:])

    nc.compile()
    return nc


_NC_CACHE = {}


def _host_prep(reference_sample, i_cnd_series, emb, w_ih, w_hh, b_ih, b_hh,
               fc1_w, fc1_b, fc2_w, fc2_b, T):
    w_ih = np.asarray(w_ih, np.float32)
    w_hh = np.asarray(w_hh, np.float32)
    b_ih = np.asarray(b_ih, np.float32)
    b_hh = np.asarray(b_hh, np.float32)
    # rz rows (0:2H) get b_hh folded into the gx bias; n rows must have
    # b_hh = 0 (r multiplies W_n@h only -- nonzero b_hh_n unsupported)
    if np.any(np.abs(b_hh[RZW:]) > 0):
        raise NotImplementedError("nonzero b_hh n-gate not supported")
    bias_row = b_ih.copy()
    bias_row[:RZW] += b_hh[:RZW]
    # column scales: rz scaled by S_TOT (enters psum), n unscaled (DVE add)
    colscale = np.concatenate([np.full(RZW, S_TOT, np.float32),
                               np.ones(THREEH - RZW, np.float32)])
    G_tab = ((np.asarray(emb, np.float32) @ w_ih[:, :EMB].T)
             + bias_row[None, :]) * colscale[None, :]
    WcT = w_ih[:, EMB:].T.copy() * colscale[None, :]
    wstack = np.concatenate([G_tab, WcT], 0).astype(BF16)
    # W_hh^T scaled, padded 896->1024, pair layout [p*128+q, i*3H+n]
    whT = w_hh.T.copy() * S_W                      # [H, 3H]
    whT = np.concatenate([whT[:, :RZW],
                          np.zeros((H, GPAD), np.float32),
                          whT[:, RZW:]], 1)        # [H, PW]
    whT = np.concatenate([whT, np.zeros((NPAIR * 256 - H, PW),
                                        np.float32)], 0)
    whhp = (whT.reshape(NPAIR, 2, 128, PW)
            .transpose(0, 2, 1, 3).reshape(NPAIR * 128, 2 * PW)
            .astype(NP8))
    fc1wt = np.asarray(fc1_w, np.float32).T.copy().astype(NP16)
    fc2wt = np.asarray(fc2_w, np.float32).T.copy().astype(NP16)
    fc1bt = np.asarray(fc1_b, np.float32).reshape(FCD // 128, 128).T.copy()
    fc2bt = np.asarray(fc2_b, np.float32).reshape(O // 128, 128).T.copy()
    eye4h = np.eye(BL, dtype=NP16)
    eye416 = np.zeros((BL, 16), np.float32)
    eye416[:, :BL] = np.eye(BL)
    eye416 = eye416.astype(BF16)
    eye128f = np.eye(128, dtype=np.float32)
    shared = dict(wstack=wstack, whhp=whhp, fc1wt=fc1wt, fc2wt=fc2wt,
                  fc1bt=fc1bt, fc2bt=fc2bt, eye4h=eye4h, eye416=eye416,
                  eye128f=eye128f)
    sample = np.asarray(reference_sample)
    cnd = np.asarray(i_cnd_series, np.float32)
    in_maps = []
    for c in range(NCORES):
        sl = slice(c * BL, (c + 1) * BL)
        idx = sample[sl, :T].T.reshape(-1).astype(np.float32)  # (T*BL,)
        idxb = np.broadcast_to(idx[None, :], (128, T * BL)).copy()
        cndr = np.ascontiguousarray(
            cnd[sl, :T].transpose(1, 0, 2)).reshape(T * BL, C2)
        in_maps.append(dict(idxb=idxb, cndr=cndr, **shared))
    return in_maps


def kernel(reference_sample, i_cnd_series, emb, w_ih, w_hh, b_ih, b_hh,
           fc1_w, fc1_b, fc2_w, fc2_b, T=None, unroll=32, dumw=0):
    T = T or np.asarray(reference_sample).shape[1]
    in_maps = _host_prep(reference_sample, i_cnd_series, emb, w_ih, w_hh,
                         b_ih, b_hh, fc1_w, fc1_b, fc2_w, fc2_b, T)
    key = (T, unroll)
    if key not in _NC_CACHE:
        _NC_CACHE[key] = build_nc(T, unroll, dumw)
    nc = _NC_CACHE[key]
    res = run_bass_kernel_spmd(nc, in_maps, core_ids=list(range(NCORES)))
    outs = []
    for c in range(NCORES):
        o = res.results[c]["outp"].reshape(T, BL, O).transpose(1, 0, 2)
        outs.append(o)
    return np.concatenate(outs, 0).astype(np.float32)
